# revision 1
# baseline (speedup 1.0000x reference)
"""Trainium2 Bass kernel for nn_InterfaceGraph (retrieval_knn).

Computes segment-restricted nearest neighbors between pos_a and pos_b
(16384 x 16384 pairwise distances, block-diagonal over 64 sorted graphs),
sharded over 8 NeuronCores (8 graphs per core).

Device work per core: for every 128-row tile of a graph block, one K=5
fp32 matmul produces -d2 = 2*a.b - |a|^2 - |b|^2 in PSUM; ScalarE copies
to SBUF; VectorE max/max_index extract the row-min distance and its
first-occurrence argmin.  Both directions (a->b and b->a) are computed.
Host does the O(N) epilogue: gather + norm (same arithmetic as the
reference), residue segment-max interface mask, mutation OR, concat.
"""

import numpy as np

NCORES = 8
G = 64            # graphs, values of node2graph
GPC = G // NCORES  # graphs per core
NUM_RESIDUES = 2048
CUTOFF = np.float32(10.0)
BIG = np.float32(2.0 ** 26)
K = 5             # contraction dim of the distance matmul

PROFILE = False       # set True (e.g. from test.py) to capture NTFF profile
LAST_EXEC_NS = None   # filled when PROFILE is True

_prog_cache = {}


def _round_up(x, m):
    return (x + m - 1) // m * m


def _install_ntff_hook():
    import sys
    import types
    if 'antenv.axon_hooks' in sys.modules:
        return
    from trn_agent_boot.trn_boot import _ntff_profile_via_ctypes
    hook = _ntff_profile_via_ctypes('/opt/axon/libaxon_pjrt.so')
    mod = types.ModuleType('antenv.axon_hooks')
    mod.get_axon_ntff_profile_hook = lambda: hook
    sys.modules['antenv.axon_hooks'] = mod


def _build_program(TA, TB, NB, NA):
    """One SPMD program shared by all 8 cores.

    TA/TB: 128-row tiles per graph (a-side / b-side).
    NB/NA: padded column width per graph block (opposing side count).
    """
    from contextlib import ExitStack

    import concourse.bacc as bacc
    import concourse.bass as bass
    import concourse.mybir as mybir
    import concourse.tile as tile

    f32 = mybir.dt.float32
    u32 = mybir.dt.uint32

    nc = bacc.Bacc("TRN2", target_bir_lowering=False, debug=False,
                   enable_asserts=True, num_devices=NCORES)

    lhsA = nc.dram_tensor("lhsA", [K, GPC * TA * 128], f32, kind="ExternalInput").ap()
    rhsB = nc.dram_tensor("rhsB", [K, GPC * NB], f32, kind="ExternalInput").ap()
    lhsB = nc.dram_tensor("lhsB", [K, GPC * TB * 128], f32, kind="ExternalInput").ap()
    rhsA = nc.dram_tensor("rhsA", [K, GPC * NA], f32, kind="ExternalInput").ap()
    valA = nc.dram_tensor("valA", [128, GPC * TA * 8], f32, kind="ExternalOutput").ap()
    idxA = nc.dram_tensor("idxA", [128, GPC * TA * 8], u32, kind="ExternalOutput").ap()
    valB = nc.dram_tensor("valB", [128, GPC * TB * 8], f32, kind="ExternalOutput").ap()
    idxB = nc.dram_tensor("idxB", [128, GPC * TB * 8], u32, kind="ExternalOutput").ap()

    with tile.TileContext(nc) as tc:
        with ExitStack() as ctx:
            const = ctx.enter_context(tc.tile_pool(name="const", bufs=1))
            psum = ctx.enter_context(
                tc.tile_pool(name="psum", bufs=8, space="PSUM"))
            work = ctx.enter_context(tc.tile_pool(name="work", bufs=4))

            lhsA_sb = const.tile([K, GPC * TA * 128], f32, tag="lhsA")
            nc.sync.dma_start(lhsA_sb[:], lhsA[:])
            rhsB_sb = const.tile([K, GPC * NB], f32, tag="rhsB")
            nc.sync.dma_start(rhsB_sb[:], rhsB[:])
            lhsB_sb = const.tile([K, GPC * TB * 128], f32, tag="lhsB")
            nc.sync.dma_start(lhsB_sb[:], lhsB[:])
            rhsA_sb = const.tile([K, GPC * NA], f32, tag="rhsA")
            nc.sync.dma_start(rhsA_sb[:], rhsA[:])

            valA_sb = const.tile([128, GPC * TA * 8], f32, tag="valA")
            idxA_sb = const.tile([128, GPC * TA * 8], u32, tag="idxA")
            valB_sb = const.tile([128, GPC * TB * 8], f32, tag="valB")
            idxB_sb = const.tile([128, GPC * TB * 8], u32, tag="idxB")

            def side(lhs_sb, rhs_sb, T, N, val_sb, idx_sb):
                for s in range(GPC):
                    for t in range(T):
                        k = s * T + t
                        ps = psum.tile([128, N], f32, tag="ps")
                        nc.tensor.matmul(
                            ps[:],
                            lhs_sb[:, k * 128:(k + 1) * 128],
                            rhs_sb[:, s * N:(s + 1) * N],
                            start=True, stop=True)
                        d2 = work.tile([128, N], f32, tag="d2")
                        nc.scalar.copy(d2[:], ps[:])
                        nc.vector.max(val_sb[:, k * 8:(k + 1) * 8], d2[:])
                        nc.vector.max_index(
                            idx_sb[:, k * 8:(k + 1) * 8],
                            val_sb[:, k * 8:(k + 1) * 8], d2[:])

            side(lhsA_sb, rhsB_sb, TA, NB, valA_sb, idxA_sb)
            side(lhsB_sb, rhsA_sb, TB, NA, valB_sb, idxB_sb)

            nc.sync.dma_start(valA[:], valA_sb[:])
            nc.sync.dma_start(idxA[:], idxA_sb[:])
            nc.sync.dma_start(valB[:], valB_sb[:])
            nc.sync.dma_start(idxB[:], idxB_sb[:])

    nc.compile()
    return nc


def _pack_side(pos_moving, pos_stationary, starts_m, starts_s, core, T, N):
    """Build lhs (stationary rows, negated-distance form) and rhs (moving
    columns) packs for one core and one direction.

    lhs rows: [2x, 2y, 2z, -1, -|p|^2] of the row-side points.
    rhs rows: [x, y, z, |q|^2, 1] of the column-side points.
    => lhs.T @ rhs = 2 p.q - |q|^2 - |p|^2 = -d2
    """
    lhs = np.zeros((K, GPC * T * 128), dtype=np.float32)
    rhs = np.zeros((K, GPC * N), dtype=np.float32)
    rhs[3, :] = BIG   # pad columns lose every argmax
    rhs[4, :] = 1.0
    for s in range(GPC):
        g = core * GPC + s
        p = pos_stationary[starts_s[g]:starts_s[g + 1]]
        n = p.shape[0]
        base = s * T * 128
        lhs[0, base:base + n] = 2.0 * p[:, 0]
        lhs[1, base:base + n] = 2.0 * p[:, 1]
        lhs[2, base:base + n] = 2.0 * p[:, 2]
        lhs[3, base:base + n] = -1.0
        sq = (p[:, 0] * p[:, 0] + p[:, 1] * p[:, 1]) + p[:, 2] * p[:, 2]
        lhs[4, base:base + n] = -sq

        q = pos_moving[starts_m[g]:starts_m[g + 1]]
        m = q.shape[0]
        base = s * N
        rhs[0, base:base + m] = q[:, 0]
        rhs[1, base:base + m] = q[:, 1]
        rhs[2, base:base + m] = q[:, 2]
        qq = (q[:, 0] * q[:, 0] + q[:, 1] * q[:, 1]) + q[:, 2] * q[:, 2]
        rhs[3, base:base + m] = qq
        rhs[4, base:base + m] = 1.0
    return lhs, rhs


def _unpack_side(res_idx, starts_s, starts_m, core, T, idx_full):
    """res_idx: [128, GPC*T*8] uint32 -> global moving-side index per
    stationary atom."""
    for s in range(GPC):
        g = core * GPC + s
        n = starts_s[g + 1] - starts_s[g]
        for t in range((n + 127) // 128):
            rows = min(128, n - t * 128)
            k = s * T + t
            loc = res_idx[:rows, k * 8].astype(np.int64)
            atoms = starts_s[g] + t * 128 + np.arange(rows)
            idx_full[atoms] = starts_m[g] + loc


def kernel(pos_a, pos_b, node2graph_a, node2graph_b,
           atom2residue_a, atom2residue_b, is_mutation):
    global LAST_EXEC_NS

    from concourse.bass_utils import run_bass_kernel_spmd

    pos_a = np.asarray(pos_a, dtype=np.float32)
    pos_b = np.asarray(pos_b, dtype=np.float32)
    node2graph_a = np.asarray(node2graph_a)
    node2graph_b = np.asarray(node2graph_b)
    atom2residue_a = np.asarray(atom2residue_a)
    atom2residue_b = np.asarray(atom2residue_b)
    is_mutation = np.asarray(is_mutation)

    Na = pos_a.shape[0]
    Nb = pos_b.shape[0]

    # graph boundaries (node2graph_* sorted)
    sa = np.searchsorted(node2graph_a, np.arange(G + 1)).astype(np.int64)
    sb = np.searchsorted(node2graph_b, np.arange(G + 1)).astype(np.int64)
    na = np.diff(sa)
    nb = np.diff(sb)
    assert na.min() > 0 and nb.min() > 0, "empty graph block not supported"

    TA = int(-(-na.max() // 128))
    TB = int(-(-nb.max() // 128))
    NB = int(max(8, _round_up(int(nb.max()), 4)))
    NA = int(max(8, _round_up(int(na.max()), 4)))

    key = (TA, TB, NB, NA)
    if key not in _prog_cache:
        _prog_cache[key] = _build_program(*key)
    nc = _prog_cache[key]

    in_maps = []
    for c in range(NCORES):
        lhsA, rhsB = _pack_side(pos_b, pos_a, sb, sa, c, TA, NB)
        lhsB, rhsA = _pack_side(pos_a, pos_b, sa, sb, c, TB, NA)
        in_maps.append({"lhsA": lhsA, "rhsB": rhsB,
                        "lhsB": lhsB, "rhsA": rhsA})

    if PROFILE:
        _install_ntff_hook()
    res = run_bass_kernel_spmd(nc, in_maps, list(range(NCORES)),
                               trace=bool(PROFILE))
    if PROFILE:
        LAST_EXEC_NS = res.exec_time_ns

    idx_a = np.zeros(Na, dtype=np.int64)   # nearest b for each a
    idx_b = np.zeros(Nb, dtype=np.int64)   # nearest a for each b
    for c in range(NCORES):
        _unpack_side(res.results[c]["idxA"], sa, sb, c, TA, idx_a)
        _unpack_side(res.results[c]["idxB"], sb, sa, c, TB, idx_b)

    # epilogue: same arithmetic as the reference
    da = pos_a - pos_b[idx_a]
    dist_a = np.sqrt((da[:, 0] * da[:, 0] + da[:, 1] * da[:, 1])
                     + da[:, 2] * da[:, 2])
    db = pos_b - pos_a[idx_b]
    dist_b = np.sqrt((db[:, 0] * db[:, 0] + db[:, 1] * db[:, 1])
                     + db[:, 2] * db[:, 2])

    def iface_mask(dist, atom2residue):
        is_if = (dist < CUTOFF).astype(np.int32)
        res_max = np.zeros(NUM_RESIDUES, dtype=np.int32)
        np.maximum.at(res_max, atom2residue, is_if)
        return res_max[atom2residue] > 0

    mask_a = iface_mask(dist_a, atom2residue_a)
    mask_b = iface_mask(dist_b, atom2residue_b)
    mask = np.concatenate([mask_a, mask_b]) | is_mutation.astype(bool)
    dists = np.concatenate([dist_a, dist_b]).astype(np.float32)
    return mask, dists


# revision 8
# speedup vs baseline: 1.3211x; 1.3211x over previous
"""Trainium2 Bass kernel for nn_InterfaceGraph (retrieval_knn).

Segment-restricted nearest neighbors between pos_a and pos_b (16384 x
16384 pairwise distances, block-diagonal over 64 sorted graphs), sharded
over 8 NeuronCores (8 graphs per core, slot-sorted by size so the SPMD
program's per-slot shapes stay tight).

Per 128-row tile of a graph block, one bf16 matmul (K=21: a bf16x3
split of 2*a.b - |b|^2, small terms accumulated first) writes the
negated-distance key into PSUM at full speed; VectorE max/max_index read
PSUM directly and produce the row min + first-occurrence argmin, exactly
matching fp32 argmin semantics to ~1-2 ulp (validated: zero flips vs the
fp32 reference on the target data).  |a|^2 is omitted: it is constant
along the scanned axis, so it cannot change the argmin.  Both directions
(a->b, b->a) are computed the same way.

Host does the O(N) epilogue: gather + norm (same arithmetic as the
reference), residue segment-max interface mask, mutation OR, concat.
"""

import numpy as np
import ml_dtypes

NCORES = 8
G = 64
GPC = G // NCORES
NUM_RESIDUES = 2048
CUTOFF = np.float32(10.0)
BIG = np.float32(2.0 ** 26)
K = 21            # 9 tier-2 + 6 tier-1 + 3 tier-0 cross rows + 3 |b|^2 rows

PROFILE = False
LAST_EXEC_NS = None

BF16 = ml_dtypes.bfloat16

_prog_cache = {}


def _round_up(x, m):
    return (x + m - 1) // m * m


def _install_ntff_hook():
    import sys
    import types
    if 'antenv.axon_hooks' in sys.modules:
        return
    from trn_agent_boot.trn_boot import _ntff_profile_via_ctypes
    hook = _ntff_profile_via_ctypes('/opt/axon/libaxon_pjrt.so')
    mod = types.ModuleType('antenv.axon_hooks')
    mod.get_axon_ntff_profile_hook = lambda: hook
    sys.modules['antenv.axon_hooks'] = mod


def _split3(v):
    """bf16x3 split: v ~= v1 + v2 + v3 with ~24-bit mantissa coverage."""
    v = v.astype(np.float32)
    v1 = v.astype(BF16).astype(np.float32)
    r = v - v1
    v2 = r.astype(BF16).astype(np.float32)
    v3 = (r - v2).astype(BF16).astype(np.float32)
    return v1, v2, v3


class _Geom:
    """Per-slot shapes shared by all cores (SPMD program is one program).

    Slot assignment is independent per side: A-side slots sort each
    core's graphs by na desc (tile count), B-side by nb desc, which
    keeps the cross-core per-slot maxima tight.
    """

    def __init__(self, na, nb):
        gid = (np.arange(NCORES * GPC).reshape(NCORES, GPC) // GPC) * GPC
        ordA = np.zeros((NCORES, GPC), dtype=np.int64)
        ordB = np.zeros((NCORES, GPC), dtype=np.int64)
        for c in range(NCORES):
            loc = np.arange(GPC)
            ordA[c] = loc[np.argsort(-na[c * GPC + loc], kind="stable")]
            ordB[c] = loc[np.argsort(-nb[c * GPC + loc], kind="stable")]
        self.graphA = gid + ordA               # [core, slot] -> graph id
        self.graphB = gid + ordB
        na_A = na[self.graphA]
        nb_A = nb[self.graphA]
        nb_B = nb[self.graphB]
        na_B = na[self.graphB]
        self.TA = [int(-(-na_A[:, s].max() // 128)) for s in range(GPC)]
        self.TB = [int(-(-nb_B[:, s].max() // 128)) for s in range(GPC)]
        self.WB = [int(max(8, _round_up(int(nb_A[:, s].max()), 4)))
                   for s in range(GPC)]
        self.WA = [int(max(8, _round_up(int(na_B[:, s].max()), 4)))
                   for s in range(GPC)]
        self.baseTA = np.concatenate([[0], np.cumsum(self.TA)]).astype(int)
        self.baseTB = np.concatenate([[0], np.cumsum(self.TB)]).astype(int)
        self.baseWB = np.concatenate([[0], np.cumsum(self.WB)]).astype(int)
        self.baseWA = np.concatenate([[0], np.cumsum(self.WA)]).astype(int)

    def key(self):
        return (tuple(self.TA), tuple(self.TB), tuple(self.WB), tuple(self.WA))


def _build_program(geom):
    from contextlib import ExitStack

    import concourse.bacc as bacc
    import concourse.mybir as mybir
    import concourse.tile as tile

    f32 = mybir.dt.float32
    bf16 = mybir.dt.bfloat16
    u32 = mybir.dt.uint32

    LA = int(geom.baseTA[-1]) * 128   # lhsA columns
    LB = int(geom.baseTB[-1]) * 128
    RB = int(geom.baseWB[-1])         # rhsB columns
    RA = int(geom.baseWA[-1])
    OA = int(geom.baseTA[-1]) * 8     # output columns, a-side
    OB = int(geom.baseTB[-1]) * 8

    nc = bacc.Bacc("TRN2", target_bir_lowering=False, debug=False,
                   enable_asserts=True, num_devices=NCORES)

    lhsA = nc.dram_tensor("lhsA", [K, LA], bf16, kind="ExternalInput").ap()
    rhsB = nc.dram_tensor("rhsB", [K, RB], bf16, kind="ExternalInput").ap()
    lhsB = nc.dram_tensor("lhsB", [K, LB], bf16, kind="ExternalInput").ap()
    rhsA = nc.dram_tensor("rhsA", [K, RA], bf16, kind="ExternalInput").ap()
    idxA = nc.dram_tensor("idxA", [128, OA], u32, kind="ExternalOutput").ap()
    idxB = nc.dram_tensor("idxB", [128, OB], u32, kind="ExternalOutput").ap()

    with tile.TileContext(nc) as tc:
        with ExitStack() as ctx:
            const = ctx.enter_context(tc.tile_pool(name="const", bufs=1))
            psum = ctx.enter_context(
                tc.tile_pool(name="psum", bufs=8, space="PSUM"))
            work = ctx.enter_context(tc.tile_pool(name="work", bufs=6))

            lhsA_sb = const.tile([K, LA], bf16, tag="lhsA")
            nc.sync.dma_start(lhsA_sb[:], lhsA[:])
            rhsB_sb = const.tile([K, RB], bf16, tag="rhsB")
            nc.sync.dma_start(rhsB_sb[:], rhsB[:])
            lhsB_sb = const.tile([K, LB], bf16, tag="lhsB")
            nc.sync.dma_start(lhsB_sb[:], lhsB[:])
            rhsA_sb = const.tile([K, RA], bf16, tag="rhsA")
            nc.sync.dma_start(rhsA_sb[:], rhsA[:])

            valA_sb = const.tile([128, OA], f32, tag="valA")
            idxA_sb = const.tile([128, OA], u32, tag="idxA")
            valB_sb = const.tile([128, OB], f32, tag="valB")
            idxB_sb = const.tile([128, OB], u32, tag="idxB")

            def side(lhs_sb, rhs_sb, T, baseT, W, baseW, val_sb, idx_sb):
                for s in range(GPC):
                    for t in range(T[s]):
                        kk = int(baseT[s]) + t
                        ps = psum.tile([128, W[s]], f32, tag="ps")
                        nc.tensor.matmul(
                            ps[:],
                            lhs_sb[:, kk * 128:(kk + 1) * 128],
                            rhs_sb[:, int(baseW[s]):int(baseW[s]) + W[s]],
                            start=True, stop=True)
                        # ScalarE copy to SBUF: VectorE SBUF reads start
                        # ~65ns faster per op than PSUM reads, and ACT is
                        # otherwise idle.
                        d2 = work.tile([128, W[s]], f32, tag="d2")
                        nc.scalar.copy(d2[:], ps[:])
                        nc.vector.max(val_sb[:, kk * 8:(kk + 1) * 8], d2[:])
                        nc.vector.max_index(
                            idx_sb[:, kk * 8:(kk + 1) * 8],
                            val_sb[:, kk * 8:(kk + 1) * 8], d2[:])

            side(lhsA_sb, rhsB_sb, geom.TA, geom.baseTA,
                 geom.WB, geom.baseWB, valA_sb, idxA_sb)
            side(lhsB_sb, rhsA_sb, geom.TB, geom.baseTB,
                 geom.WA, geom.baseWA, valB_sb, idxB_sb)

            nc.sync.dma_start(idxA[:], idxA_sb[:])
            nc.sync.dma_start(idxB[:], idxB_sb[:])

    nc.compile()
    return nc


def _pack_side(pos_row, pos_col, starts_row, starts_col, graphs,
               T, baseT, W, baseW):
    """lhs/rhs bf16 packs for one core, one direction.

    Row side (stationary): coords doubled, bf16x3 split.
    Col side (moving): coords + |q|^2 split; key = 2 p.q - |q|^2.
    K-row order: tier-2 (smallest) first, tier-0 last.
    """
    LT = int(baseT[-1]) * 128
    RW = int(baseW[-1])
    lhs = np.zeros((K, LT), dtype=np.float32)
    rhs = np.zeros((K, RW), dtype=np.float32)
    # q-split rows: tier2 row 9, tier1 rows 15-16?  layout below:
    #  rows 0-8   : tier2 cross (c,x3) lhs a1,a2,a3 / rhs b3,b2,b1
    #  row  9     : tier2 -q3      (lhs -1, rhs q3)
    #  rows 10-15 : tier1 cross    lhs a1,a2 / rhs b2,b1
    #  row  16    : tier1 -q2
    #  rows 17-19 : tier0 cross    lhs a1 / rhs b1
    #  row  20    : tier0 -q1  (+BIG on padding)
    lhs[9, :] = -1.0
    lhs[16, :] = -1.0
    lhs[20, :] = -1.0
    rhs[20, :] = BIG  # padding columns lose every argmax
    for s in range(GPC):
        g = graphs[s]
        p = pos_row[starts_row[g]:starts_row[g + 1]]
        n = p.shape[0]
        lb = int(baseT[s]) * 128
        for c in range(3):
            a1, a2, a3 = _split3(np.float32(2.0) * p[:, c])
            lhs[0 + c * 3, lb:lb + n] = a1
            lhs[1 + c * 3, lb:lb + n] = a2
            lhs[2 + c * 3, lb:lb + n] = a3
            lhs[10 + c * 2, lb:lb + n] = a1
            lhs[11 + c * 2, lb:lb + n] = a2
            lhs[17 + c, lb:lb + n] = a1
        # padding rows: zero coords, and kill the -1 rows so pad rows
        # read 0 - (-BIG)?  (pad rows' outputs are discarded anyway)

        q = pos_col[starts_col[g]:starts_col[g + 1]]
        m = q.shape[0]
        rb = int(baseW[s])
        qq = (q[:, 0] * q[:, 0] + q[:, 1] * q[:, 1]) + q[:, 2] * q[:, 2]
        q1, q2, q3 = _split3(qq)
        for c in range(3):
            b1, b2, b3 = _split3(q[:, c])
            rhs[0 + c * 3, rb:rb + m] = b3
            rhs[1 + c * 3, rb:rb + m] = b2
            rhs[2 + c * 3, rb:rb + m] = b1
            rhs[10 + c * 2, rb:rb + m] = b2
            rhs[11 + c * 2, rb:rb + m] = b1
            rhs[17 + c, rb:rb + m] = b1
        rhs[9, rb:rb + m] = q3
        rhs[16, rb:rb + m] = q2
        rhs[20, rb:rb + m] = q1
    return lhs.astype(BF16), rhs.astype(BF16)


def _unpack_side(res_idx, starts_row, starts_col, graphs, baseT, idx_full):
    for s in range(GPC):
        g = graphs[s]
        n = starts_row[g + 1] - starts_row[g]
        for t in range((n + 127) // 128):
            rows = min(128, n - t * 128)
            kk = int(baseT[s]) + t
            loc = res_idx[:rows, kk * 8].astype(np.int64)
            atoms = starts_row[g] + t * 128 + np.arange(rows)
            idx_full[atoms] = starts_col[g] + loc


def kernel(pos_a, pos_b, node2graph_a, node2graph_b,
           atom2residue_a, atom2residue_b, is_mutation):
    global LAST_EXEC_NS

    from concourse.bass_utils import run_bass_kernel_spmd

    pos_a = np.asarray(pos_a, dtype=np.float32)
    pos_b = np.asarray(pos_b, dtype=np.float32)
    node2graph_a = np.asarray(node2graph_a)
    node2graph_b = np.asarray(node2graph_b)
    atom2residue_a = np.asarray(atom2residue_a)
    atom2residue_b = np.asarray(atom2residue_b)
    is_mutation = np.asarray(is_mutation)

    Na = pos_a.shape[0]
    Nb = pos_b.shape[0]

    sa = np.searchsorted(node2graph_a, np.arange(G + 1)).astype(np.int64)
    sb = np.searchsorted(node2graph_b, np.arange(G + 1)).astype(np.int64)
    na = np.diff(sa)
    nb = np.diff(sb)
    assert na.min() > 0 and nb.min() > 0, "empty graph block not supported"

    geom = _Geom(na, nb)
    key = geom.key()
    if key not in _prog_cache:
        _prog_cache[key] = _build_program(geom)
    nc = _prog_cache[key]

    in_maps = []
    for c in range(NCORES):
        lhsA, rhsB = _pack_side(pos_a, pos_b, sa, sb, geom.graphA[c],
                                geom.TA, geom.baseTA, geom.WB, geom.baseWB)
        lhsB, rhsA = _pack_side(pos_b, pos_a, sb, sa, geom.graphB[c],
                                geom.TB, geom.baseTB, geom.WA, geom.baseWA)
        in_maps.append({"lhsA": lhsA, "rhsB": rhsB,
                        "lhsB": lhsB, "rhsA": rhsA})

    if PROFILE:
        _install_ntff_hook()
    res = run_bass_kernel_spmd(nc, in_maps, list(range(NCORES)),
                               trace=bool(PROFILE))
    if PROFILE:
        LAST_EXEC_NS = res.exec_time_ns

    idx_a = np.zeros(Na, dtype=np.int64)
    idx_b = np.zeros(Nb, dtype=np.int64)
    for c in range(NCORES):
        _unpack_side(res.results[c]["idxA"], sa, sb, geom.graphA[c],
                     geom.baseTA, idx_a)
        _unpack_side(res.results[c]["idxB"], sb, sa, geom.graphB[c],
                     geom.baseTB, idx_b)

    da = pos_a - pos_b[idx_a]
    dist_a = np.sqrt((da[:, 0] * da[:, 0] + da[:, 1] * da[:, 1])
                     + da[:, 2] * da[:, 2])
    db = pos_b - pos_a[idx_b]
    dist_b = np.sqrt((db[:, 0] * db[:, 0] + db[:, 1] * db[:, 1])
                     + db[:, 2] * db[:, 2])

    def iface_mask(dist, atom2residue):
        is_if = (dist < CUTOFF).astype(np.int32)
        res_max = np.zeros(NUM_RESIDUES, dtype=np.int32)
        np.maximum.at(res_max, atom2residue, is_if)
        return res_max[atom2residue] > 0

    mask_a = iface_mask(dist_a, atom2residue_a)
    mask_b = iface_mask(dist_b, atom2residue_b)
    mask = np.concatenate([mask_a, mask_b]) | is_mutation.astype(bool)
    dists = np.concatenate([dist_a, dist_b]).astype(np.float32)
    return mask, dists


# revision 9
# speedup vs baseline: 1.3397x; 1.0141x over previous
"""Trainium2 Bass kernel for nn_InterfaceGraph (retrieval_knn).

Segment-restricted nearest neighbors between pos_a and pos_b (16384 x
16384 pairwise distances, block-diagonal over 64 sorted graphs), sharded
over 8 NeuronCores (8 graphs per core, slot-sorted by size so the SPMD
program's per-slot shapes stay tight).

Per 128-row tile of a graph block, one bf16 matmul (K=21: a bf16x3
split of 2*a.b - |b|^2, small terms accumulated first) writes the
negated-distance key into PSUM at full speed; VectorE max/max_index read
PSUM directly and produce the row min + first-occurrence argmin, exactly
matching fp32 argmin semantics to ~1-2 ulp (validated: zero flips vs the
fp32 reference on the target data).  |a|^2 is omitted: it is constant
along the scanned axis, so it cannot change the argmin.  Both directions
(a->b, b->a) are computed the same way.

Host does the O(N) epilogue: gather + norm (same arithmetic as the
reference), residue segment-max interface mask, mutation OR, concat.
"""

import numpy as np
import ml_dtypes

NCORES = 8
G = 64
GPC = G // NCORES
NUM_RESIDUES = 2048
CUTOFF = np.float32(10.0)
BIG = np.float32(2.0 ** 26)
K = 21            # 9 tier-2 + 6 tier-1 + 3 tier-0 cross rows + 3 |b|^2 rows

PROFILE = False
LAST_EXEC_NS = None

BF16 = ml_dtypes.bfloat16

_prog_cache = {}


def _round_up(x, m):
    return (x + m - 1) // m * m


def _install_ntff_hook():
    import sys
    import types
    if 'antenv.axon_hooks' in sys.modules:
        return
    from trn_agent_boot.trn_boot import _ntff_profile_via_ctypes
    hook = _ntff_profile_via_ctypes('/opt/axon/libaxon_pjrt.so')
    mod = types.ModuleType('antenv.axon_hooks')
    mod.get_axon_ntff_profile_hook = lambda: hook
    sys.modules['antenv.axon_hooks'] = mod


def _split3(v):
    """bf16x3 split: v ~= v1 + v2 + v3 with ~24-bit mantissa coverage."""
    v = v.astype(np.float32)
    v1 = v.astype(BF16).astype(np.float32)
    r = v - v1
    v2 = r.astype(BF16).astype(np.float32)
    v3 = (r - v2).astype(BF16).astype(np.float32)
    return v1, v2, v3


class _Geom:
    """Per-slot shapes shared by all cores (SPMD program is one program).

    Slot assignment is independent per side: A-side slots sort each
    core's graphs by na desc (tile count), B-side by nb desc, which
    keeps the cross-core per-slot maxima tight.
    """

    def __init__(self, na, nb):
        gid = (np.arange(NCORES * GPC).reshape(NCORES, GPC) // GPC) * GPC
        ordA = np.zeros((NCORES, GPC), dtype=np.int64)
        ordB = np.zeros((NCORES, GPC), dtype=np.int64)
        for c in range(NCORES):
            loc = np.arange(GPC)
            ordA[c] = loc[np.argsort(-na[c * GPC + loc], kind="stable")]
            ordB[c] = loc[np.argsort(-nb[c * GPC + loc], kind="stable")]
        self.graphA = gid + ordA               # [core, slot] -> graph id
        self.graphB = gid + ordB
        na_A = na[self.graphA]
        nb_A = nb[self.graphA]
        nb_B = nb[self.graphB]
        na_B = na[self.graphB]
        self.TA = [int(-(-na_A[:, s].max() // 128)) for s in range(GPC)]
        self.TB = [int(-(-nb_B[:, s].max() // 128)) for s in range(GPC)]
        self.WB = [int(max(8, _round_up(int(nb_A[:, s].max()), 4)))
                   for s in range(GPC)]
        self.WA = [int(max(8, _round_up(int(na_B[:, s].max()), 4)))
                   for s in range(GPC)]
        self.baseTA = np.concatenate([[0], np.cumsum(self.TA)]).astype(int)
        self.baseTB = np.concatenate([[0], np.cumsum(self.TB)]).astype(int)
        self.baseWB = np.concatenate([[0], np.cumsum(self.WB)]).astype(int)
        self.baseWA = np.concatenate([[0], np.cumsum(self.WA)]).astype(int)

    def key(self):
        return (tuple(self.TA), tuple(self.TB), tuple(self.WB), tuple(self.WA))


def _build_program(geom):
    from contextlib import ExitStack

    import concourse.bacc as bacc
    import concourse.mybir as mybir
    import concourse.tile as tile

    f32 = mybir.dt.float32
    bf16 = mybir.dt.bfloat16
    u32 = mybir.dt.uint32

    LA = int(geom.baseTA[-1]) * 128   # lhsA columns
    LB = int(geom.baseTB[-1]) * 128
    RB = int(geom.baseWB[-1])         # rhsB columns
    RA = int(geom.baseWA[-1])
    OA = int(geom.baseTA[-1]) * 8     # output columns, a-side
    OB = int(geom.baseTB[-1]) * 8

    nc = bacc.Bacc("TRN2", target_bir_lowering=False, debug=False,
                   enable_asserts=True, num_devices=NCORES)

    lhsA = nc.dram_tensor("lhsA", [K, LA], bf16, kind="ExternalInput").ap()
    rhsB = nc.dram_tensor("rhsB", [K, RB], bf16, kind="ExternalInput").ap()
    lhsB = nc.dram_tensor("lhsB", [K, LB], bf16, kind="ExternalInput").ap()
    rhsA = nc.dram_tensor("rhsA", [K, RA], bf16, kind="ExternalInput").ap()
    idxA = nc.dram_tensor("idxA", [128, OA], u32, kind="ExternalOutput").ap()
    idxB = nc.dram_tensor("idxB", [128, OB], u32, kind="ExternalOutput").ap()

    with tile.TileContext(nc) as tc:
        with ExitStack() as ctx:
            const = ctx.enter_context(tc.tile_pool(name="const", bufs=1))
            psum = ctx.enter_context(
                tc.tile_pool(name="psum", bufs=8, space="PSUM"))
            work = ctx.enter_context(tc.tile_pool(name="work", bufs=6))

            lhsA_sb = const.tile([K, LA], bf16, tag="lhsA")
            nc.sync.dma_start(lhsA_sb[:], lhsA[:])
            rhsB_sb = const.tile([K, RB], bf16, tag="rhsB")
            nc.sync.dma_start(rhsB_sb[:], rhsB[:])
            lhsB_sb = const.tile([K, LB], bf16, tag="lhsB")
            nc.sync.dma_start(lhsB_sb[:], lhsB[:])
            rhsA_sb = const.tile([K, RA], bf16, tag="rhsA")
            nc.sync.dma_start(rhsA_sb[:], rhsA[:])

            valA_sb = const.tile([128, OA], f32, tag="valA")
            idxA_sb = const.tile([128, OA], u32, tag="idxA")
            valB_sb = const.tile([128, OB], f32, tag="valB")
            idxB_sb = const.tile([128, OB], u32, tag="idxB")

            def side(lhs_sb, rhs_sb, T, baseT, W, baseW, val_sb, idx_sb):
                for s in range(GPC):
                    for t in range(T[s]):
                        kk = int(baseT[s]) + t
                        ps = psum.tile([128, W[s]], f32, tag="ps")
                        nc.tensor.matmul(
                            ps[:],
                            lhs_sb[:, kk * 128:(kk + 1) * 128],
                            rhs_sb[:, int(baseW[s]):int(baseW[s]) + W[s]],
                            start=True, stop=True)
                        # VectorE max/max_index read PSUM directly (measured
                        # same per-op cost as SBUF; skipping the ScalarE
                        # copy shortens each tile's dependency chain).
                        nc.vector.max(val_sb[:, kk * 8:(kk + 1) * 8], ps[:])
                        nc.vector.max_index(
                            idx_sb[:, kk * 8:(kk + 1) * 8],
                            val_sb[:, kk * 8:(kk + 1) * 8], ps[:])

            side(lhsA_sb, rhsB_sb, geom.TA, geom.baseTA,
                 geom.WB, geom.baseWB, valA_sb, idxA_sb)
            side(lhsB_sb, rhsA_sb, geom.TB, geom.baseTB,
                 geom.WA, geom.baseWA, valB_sb, idxB_sb)

            nc.sync.dma_start(idxA[:], idxA_sb[:])
            nc.sync.dma_start(idxB[:], idxB_sb[:])

    nc.compile()
    return nc


def _pack_side(pos_row, pos_col, starts_row, starts_col, graphs,
               T, baseT, W, baseW):
    """lhs/rhs bf16 packs for one core, one direction.

    Row side (stationary): coords doubled, bf16x3 split.
    Col side (moving): coords + |q|^2 split; key = 2 p.q - |q|^2.
    K-row order: tier-2 (smallest) first, tier-0 last.
    """
    LT = int(baseT[-1]) * 128
    RW = int(baseW[-1])
    lhs = np.zeros((K, LT), dtype=np.float32)
    rhs = np.zeros((K, RW), dtype=np.float32)
    # q-split rows: tier2 row 9, tier1 rows 15-16?  layout below:
    #  rows 0-8   : tier2 cross (c,x3) lhs a1,a2,a3 / rhs b3,b2,b1
    #  row  9     : tier2 -q3      (lhs -1, rhs q3)
    #  rows 10-15 : tier1 cross    lhs a1,a2 / rhs b2,b1
    #  row  16    : tier1 -q2
    #  rows 17-19 : tier0 cross    lhs a1 / rhs b1
    #  row  20    : tier0 -q1  (+BIG on padding)
    lhs[9, :] = -1.0
    lhs[16, :] = -1.0
    lhs[20, :] = -1.0
    rhs[20, :] = BIG  # padding columns lose every argmax
    for s in range(GPC):
        g = graphs[s]
        p = pos_row[starts_row[g]:starts_row[g + 1]]
        n = p.shape[0]
        lb = int(baseT[s]) * 128
        for c in range(3):
            a1, a2, a3 = _split3(np.float32(2.0) * p[:, c])
            lhs[0 + c * 3, lb:lb + n] = a1
            lhs[1 + c * 3, lb:lb + n] = a2
            lhs[2 + c * 3, lb:lb + n] = a3
            lhs[10 + c * 2, lb:lb + n] = a1
            lhs[11 + c * 2, lb:lb + n] = a2
            lhs[17 + c, lb:lb + n] = a1
        # padding rows: zero coords, and kill the -1 rows so pad rows
        # read 0 - (-BIG)?  (pad rows' outputs are discarded anyway)

        q = pos_col[starts_col[g]:starts_col[g + 1]]
        m = q.shape[0]
        rb = int(baseW[s])
        qq = (q[:, 0] * q[:, 0] + q[:, 1] * q[:, 1]) + q[:, 2] * q[:, 2]
        q1, q2, q3 = _split3(qq)
        for c in range(3):
            b1, b2, b3 = _split3(q[:, c])
            rhs[0 + c * 3, rb:rb + m] = b3
            rhs[1 + c * 3, rb:rb + m] = b2
            rhs[2 + c * 3, rb:rb + m] = b1
            rhs[10 + c * 2, rb:rb + m] = b2
            rhs[11 + c * 2, rb:rb + m] = b1
            rhs[17 + c, rb:rb + m] = b1
        rhs[9, rb:rb + m] = q3
        rhs[16, rb:rb + m] = q2
        rhs[20, rb:rb + m] = q1
    return lhs.astype(BF16), rhs.astype(BF16)


def _unpack_side(res_idx, starts_row, starts_col, graphs, baseT, idx_full):
    for s in range(GPC):
        g = graphs[s]
        n = starts_row[g + 1] - starts_row[g]
        for t in range((n + 127) // 128):
            rows = min(128, n - t * 128)
            kk = int(baseT[s]) + t
            loc = res_idx[:rows, kk * 8].astype(np.int64)
            atoms = starts_row[g] + t * 128 + np.arange(rows)
            idx_full[atoms] = starts_col[g] + loc


def kernel(pos_a, pos_b, node2graph_a, node2graph_b,
           atom2residue_a, atom2residue_b, is_mutation):
    global LAST_EXEC_NS

    from concourse.bass_utils import run_bass_kernel_spmd

    pos_a = np.asarray(pos_a, dtype=np.float32)
    pos_b = np.asarray(pos_b, dtype=np.float32)
    node2graph_a = np.asarray(node2graph_a)
    node2graph_b = np.asarray(node2graph_b)
    atom2residue_a = np.asarray(atom2residue_a)
    atom2residue_b = np.asarray(atom2residue_b)
    is_mutation = np.asarray(is_mutation)

    Na = pos_a.shape[0]
    Nb = pos_b.shape[0]

    sa = np.searchsorted(node2graph_a, np.arange(G + 1)).astype(np.int64)
    sb = np.searchsorted(node2graph_b, np.arange(G + 1)).astype(np.int64)
    na = np.diff(sa)
    nb = np.diff(sb)
    assert na.min() > 0 and nb.min() > 0, "empty graph block not supported"

    geom = _Geom(na, nb)
    key = geom.key()
    if key not in _prog_cache:
        _prog_cache[key] = _build_program(geom)
    nc = _prog_cache[key]

    in_maps = []
    for c in range(NCORES):
        lhsA, rhsB = _pack_side(pos_a, pos_b, sa, sb, geom.graphA[c],
                                geom.TA, geom.baseTA, geom.WB, geom.baseWB)
        lhsB, rhsA = _pack_side(pos_b, pos_a, sb, sa, geom.graphB[c],
                                geom.TB, geom.baseTB, geom.WA, geom.baseWA)
        in_maps.append({"lhsA": lhsA, "rhsB": rhsB,
                        "lhsB": lhsB, "rhsA": rhsA})

    if PROFILE:
        _install_ntff_hook()
    res = run_bass_kernel_spmd(nc, in_maps, list(range(NCORES)),
                               trace=bool(PROFILE))
    if PROFILE:
        LAST_EXEC_NS = res.exec_time_ns

    idx_a = np.zeros(Na, dtype=np.int64)
    idx_b = np.zeros(Nb, dtype=np.int64)
    for c in range(NCORES):
        _unpack_side(res.results[c]["idxA"], sa, sb, geom.graphA[c],
                     geom.baseTA, idx_a)
        _unpack_side(res.results[c]["idxB"], sb, sa, geom.graphB[c],
                     geom.baseTB, idx_b)

    da = pos_a - pos_b[idx_a]
    dist_a = np.sqrt((da[:, 0] * da[:, 0] + da[:, 1] * da[:, 1])
                     + da[:, 2] * da[:, 2])
    db = pos_b - pos_a[idx_b]
    dist_b = np.sqrt((db[:, 0] * db[:, 0] + db[:, 1] * db[:, 1])
                     + db[:, 2] * db[:, 2])

    def iface_mask(dist, atom2residue):
        is_if = (dist < CUTOFF).astype(np.int32)
        res_max = np.zeros(NUM_RESIDUES, dtype=np.int32)
        np.maximum.at(res_max, atom2residue, is_if)
        return res_max[atom2residue] > 0

    mask_a = iface_mask(dist_a, atom2residue_a)
    mask_b = iface_mask(dist_b, atom2residue_b)
    mask = np.concatenate([mask_a, mask_b]) | is_mutation.astype(bool)
    dists = np.concatenate([dist_a, dist_b]).astype(np.float32)
    return mask, dists
